# revision 47
# baseline (speedup 1.0000x reference)
# Trainium2 Bass kernel for nn_CNN3_F_P (pairwise conv + 3x conv1d + 2 FC).
# Data parallel over 8 NeuronCores: batch 2048 -> 256 samples/core.
# Self-contained: hardcodes all shapes; host preps DMA-friendly weight layouts.
import sys

import numpy as np

try:
    import concourse.bass as bass  # noqa: F401
except ImportError:
    sys.path.insert(0, "/opt/trn_rl_repo")

import ml_dtypes

import concourse.bass as bass
import concourse.mybir as mybir
import concourse.tile as tile
from concourse import bacc
from concourse.bass_utils import run_bass_kernel_spmd

# Problem shapes
INST, CTX = 64, 128
PC = 256          # pairwise out channels; CH1=CH2=CH3=256
L = CTX - 1       # 127
F1, OUT = 400, 2
B = 2048
N_CORES = 8
BPC = B // N_CORES  # 256 samples per core
GT = 4              # samples per matmul group (free dim GT*L = 508 <= 512)
CH_S = 16           # samples per contiguous x DMA chunk (4 groups)
WL = 8              # fc1 weight l-values per DMA chunk

FP32 = mybir.dt.float32
BF16 = mybir.dt.bfloat16
BF16_NP = ml_dtypes.bfloat16
RELU = mybir.ActivationFunctionType.Relu
ADD = mybir.AluOpType.add
MULT = mybir.AluOpType.mult


def build_nc(n_samples: int) -> bass.Bass:
    """Emit the per-core Tile program. Every core runs this same program on
    its own 'n_samples'-sample shard."""
    assert n_samples % (2 * GT) == 0
    n_groups = n_samples // GT
    n_chunks = n_samples // CH_S
    n_wl = (L + WL - 1) // WL  # 32 fc1 weight chunks (last holds 3 valid l)
    sb_n = n_samples // 128 if n_samples >= 128 else 1
    sb_sz = min(n_samples, 128)

    nc = bacc.Bacc()

    # DRAM parameters (per-core shard + replicated weights).
    # xt chunks: [chunk, part, pair, sample, pos] with part = 64*(g%2)+inst:
    # each 16-sample chunk holds 2 group-pairs; the even group of a pair sits
    # on partitions 0..63, the odd group on 64..127, so one pair of row-tiled
    # K=64 matmuls (concurrent on the two PE array halves) does the tap-i
    # contraction for both groups at once.
    xt_d = nc.declare_dram_parameter(
        "xt", [n_chunks, 128, 2, GT, CTX], BF16, isOutput=False
    )
    # y0[p, o, sign, b]: host-precomputed pairwise pos-0 term (+bias), and its
    # negation, for h0 = max(yi, -y0') + y0'
    y0_d = nc.declare_dram_parameter("y0", [128, 2, 2, n_samples], BF16, isOutput=False)
    wp_d = nc.declare_dram_parameter("wp", [128, PC], BF16, isOutput=False)
    wcv_d = nc.declare_dram_parameter("wcv", [128, 3, 2, 3, 2, 128], BF16, isOutput=False)
    wf1_d = nc.declare_dram_parameter("wf1", [n_wl, 128, WL, 2, F1], BF16, isOutput=False)
    bf1_d = nc.declare_dram_parameter("bf1", [1, F1], BF16, isOutput=False)
    wf2_d = nc.declare_dram_parameter("wf2", [128, OUT, F1], BF16, isOutput=False)
    bcv_d = nc.declare_dram_parameter("bcv", [128, 8], FP32, isOutput=False)
    bf2_d = nc.declare_dram_parameter("bf2", [128, OUT], FP32, isOutput=False)
    # out[p, sb, o] = result for sample sb*128+p (host re-interleaves); one
    # tile + one DMA so both sample blocks share a single completion receipt
    out_d = nc.declare_dram_parameter("out", [sb_sz, sb_n, OUT], FP32, isOutput=True)
    import os
    debug = os.environ.get("KDBG", "0") == "1"
    if debug:
        dbg_yi_d = nc.declare_dram_parameter("dbg_yi", [2, 2, 128, GT, L], FP32, isOutput=True)
        dbg_h0_d = nc.declare_dram_parameter("dbg_h0", [2, 2, 128, GT, 129], FP32, isOutput=True)

    with tile.TileContext(nc) as tc:
        with (
            tc.tile_pool(name="consts", bufs=1) as consts,
            tc.tile_pool(name="hbuf", bufs=1) as hbuf,
            tc.tile_pool(name="xin", bufs=2) as xin,
            tc.tile_pool(name="wstream", bufs=3) as ws,
        ):
            # ---- resident weights/biases ----
            wp_t = consts.tile([128, PC], BF16, tag="wp", name="wp")
            y0_t = consts.tile([128, 2, 2, n_samples], BF16, tag="y0", name="y0")
            bcv_t = consts.tile([128, 8], FP32, tag="bcv", name="bcv")
            wcv_t = consts.tile([128, 3, 2, 3, 2, 128], BF16, tag="wcv", name="wcv")
            wf2_t = consts.tile([128, OUT, F1], BF16, tag="wf2", name="wf2")
            bf1_t = consts.tile([1, F1], BF16, tag="bf1", name="bf1")
            bf2_t = consts.tile([128, OUT], FP32, tag="bf2", name="bf2")
            ones_t = consts.tile([1, 128], BF16, tag="ones", name="ones")

            # x chunk streaming. HBM is the startup bottleneck (all 8 cores
            # pull their replicated weights at once), so the first bytes are
            # strictly need-ordered: chunk 0 is split so group 0 (gates the
            # first matmul) rides alone on the sync ring, the rest on gpsimd;
            # chunk 1 rides the scalar ring BEHIND the conv weights; chunks
            # >=2 go on sync, self-paced by the pool's buffer recycling.
            px_tiles = {}

            def ensure_chunk(c):
                if c in px_tiles or c >= n_chunks:
                    return
                t = xin.tile([128, 2, GT, CTX], BF16, tag="px", name="px")
                if c == 0:
                    nc.sync.dma_start(t[:, 0:1], xt_d[c, :, 0:1])
                    nc.gpsimd.dma_start(t[:, 1:2], xt_d[c, :, 1:2])
                elif c == 1:
                    nc.scalar.dma_start(t[:], xt_d[c])
                else:
                    nc.sync.dma_start(t[:], xt_d[c])
                px_tiles[c] = t

            ensure_chunk(0)
            nc.scalar.dma_start(wp_t[:], wp_d[:])
            nc.scalar.dma_start(y0_t[:], y0_d[:])
            nc.scalar.dma_start(wcv_t[:, 0], wcv_d[:, 0])
            nc.scalar.dma_start(bcv_t[:], bcv_d[:])
            nc.vector.memset(ones_t[:], 1.0)

            # ---- persistent activation buffers ----
            # h0..h2: ping-pong per group parity; stripes of 129 cols/sample
            # (col 0 and col 128 are zero pads for the k=3 conv taps).
            # h0 ring is 4 deep: pair_block writes land up to 3 groups ahead
            # of their conv1 read
            hconv = []  # hconv[layer][parity][blk]
            NPAR = [4, 2, 2]
            for layer in range(3):
                byp = []
                for par in range(NPAR[layer]):
                    blks = []
                    for o in range(2):
                        t = hbuf.tile(
                            [128, GT, 129], BF16,
                            tag=f"h{layer}_{par}_{o}", name=f"h{layer}_{par}_{o}",
                        )
                        nc.vector.memset(t[:, :, 0:1], 0.0)
                        nc.vector.memset(t[:, :, 128:129], 0.0)
                        blks.append(t)
                    byp.append(blks)
                hconv.append(byp)
            # h3: conv3 output, transposed [c, l, sample] so fc1's stationary
            # slices are contiguous, bf16
            h3 = [
                hbuf.tile([128, L, n_samples], BF16, tag=f"h3_{o}", name=f"h3_{o}")
                for o in range(2)
            ]

            # fc1 weight stream: chunks of WL l-values, prefetched behind the
            # conv phase (pool bufs bound the lookahead).
            wt_tiles = {}

            def ensure_wchunk(c):
                if c in wt_tiles or c >= n_wl:
                    return
                t = ws.tile([128, WL, 2, F1], BF16, tag="wf1", name="wf1_t")
                nc.scalar.dma_start(t[:], wf1_d[c])
                wt_tiles[c] = t

            # ---- phase A: pairwise + conv1..conv3 ----
            # Pairwise for group g+2 is emitted ahead of group g's convs so the
            # PE never waits on the h0 relu; relus alternate Scalar (o=0) and
            # Vector (o=1) so both channel blocks finish in parallel.
            MAXALU = mybir.AluOpType.max

            def relu_to(dst, ps, bias_idx, use_dve):
                if use_dve:
                    nc.vector.tensor_scalar(
                        dst, ps, bcv_t[:, bias_idx : bias_idx + 1], 0.0, ADD, MAXALU
                    )
                else:
                    nc.scalar.activation(
                        dst, ps, RELU, bias=bcv_t[:, bias_idx : bias_idx + 1]
                    )

            with (
                tc.tile_pool(name="cpsum", bufs=6, space=bass.MemorySpace.PSUM) as cp,
                tc.tile_pool(
                    name="ppsum", bufs=2, space=bass.MemorySpace.PSUM, side="right"
                ) as pp,
            ):

                def pair_block(k, o):
                    # pair k = groups (2k, 2k+1); one cout block. Two K=64
                    # matmuls on the top/bottom halves of the PE array run
                    # concurrently (distinct row groups), then each group's
                    # h0 is assembled as max(yi, -y0') + y0' on DVE.
                    c, pk = divmod(k, 2)
                    px = px_tiles[c]
                    pss = []
                    for half in range(2):
                        ps = pp.tile([128, GT, L], FP32, tag="pp", name="pp")
                        nc.tensor.matmul(
                            ps[:],
                            wp_t[half * 64 : (half + 1) * 64, o * 128 : (o + 1) * 128],
                            px[half * 64 : (half + 1) * 64, pk, :, 1:CTX],
                            start=True,
                            stop=True,
                        )
                        pss.append(ps)
                    if debug and k == 0:
                        for half in range(2):
                            dt = consts.tile(
                                [128, GT, L], FP32, tag="dyi", name="dyi", bufs=1
                            )
                            nc.vector.tensor_scalar(
                                dt[:], pss[half][:], 0.0, None, ADD
                            )
                            nc.sync.dma_start(dbg_yi_d[half, o], dt[:])
                    for half in range(2):
                        g = 2 * k + half
                        par = g % 4
                        s0 = g * GT
                        neg = (
                            y0_t[:, o, 1, s0 : s0 + GT]
                            .unsqueeze(2)
                            .broadcast_to([128, GT, L])
                        )
                        pos = (
                            y0_t[:, o, 0, s0 : s0 + GT]
                            .unsqueeze(2)
                            .broadcast_to([128, GT, L])
                        )
                        ps = pss[half]
                        nc.vector.tensor_tensor(ps[:], ps[:], neg, MAXALU)
                        nc.vector.tensor_tensor(
                            hconv[0][par][o][:, :, 1:128], ps[:], pos, ADD
                        )
                        if debug and k == 0:
                            dh = consts.tile(
                                [128, GT, 129], FP32, tag="dh", name="dh", bufs=1
                            )
                            nc.vector.tensor_scalar(
                                dh[:], hconv[0][par][o][:], 0.0, None, ADD
                            )
                            nc.sync.dma_start(dbg_h0_d[half, o], dh[:])

                def conv_layer(li, g):
                    rpar = g % 4 if li == 0 else g % 2
                    wpar = g % 2
                    s0 = g * GT
                    for o in range(2):
                        ps = cp.tile([128, GT, L], FP32, tag="cp", name="cp")
                        n_mm = 0
                        for i in range(2):
                            for k in range(3):
                                nc.tensor.matmul(
                                    ps[:],
                                    wcv_t[:, li, i, k, o, :],
                                    hconv[li][rpar][i][:, :, k : k + L],
                                    start=(n_mm == 0),
                                    stop=(n_mm == 5),
                                )
                                n_mm += 1
                        if li < 2:
                            dst = hconv[li + 1][wpar][o][:, :, 1:128]
                            src_ap = ps[:]
                        else:
                            dst = h3[o][:, :, s0 : s0 + GT]
                            src_ap = ps[:].transpose([0, 2, 1])
                        relu_to(dst, src_ap, 2 * (li + 1) + o, o == 1)

                pair_block(0, 0)
                for li in (1, 2):
                    nc.scalar.dma_start(wcv_t[:, li], wcv_d[:, li])
                ensure_chunk(1)
                pair_block(0, 1)
                for g in range(n_groups):
                    if g == 2:
                        # phase-B consts, once the startup burst has drained
                        nc.scalar.dma_start(wf2_t[:], wf2_d[:])
                        nc.scalar.dma_start(bf1_t[:], bf1_d[:])
                        nc.scalar.dma_start(bf2_t[:], bf2_d[:])
                    if 30 <= g < 42 and g % 4 == 2:
                        ensure_wchunk((g - 30) // 4)
                    gg = g + 2
                    if gg < n_groups:
                        ensure_chunk((gg * GT) // CH_S)
                        # pair gg//2: cout block 0 when gg is even, block 1
                        # one group later
                        pair_block(gg // 2, gg % 2)
                    for li in range(3):
                        conv_layer(li, g)

            # ---- phase B: fc1 (+relu) and fc2 ----
            # fc1 runs "flipped": stationary = h3 sample-block columns,
            # moving = streamed Wfc1 rows -> psum[sample, f1]. The psum pool
            # sits on the right-side banks vacated early by the pairwise pool.
            with (
                tc.tile_pool(
                    name="fpsum", bufs=1, space=bass.MemorySpace.PSUM, side="right"
                ) as fp,
                tc.tile_pool(name="fout", bufs=1) as fo,
            ):
                f1ps = [
                    fp.tile([sb_sz, F1], FP32, tag=f"f1p{sb}", name=f"f1p{sb}")
                    for sb in range(sb_n)
                ]
                # bias row via a K=1 matmul of ones^T x bfc1
                for sb in range(sb_n):
                    nc.tensor.matmul(
                        f1ps[sb][:],
                        ones_t[:, :sb_sz],
                        bf1_t[:],
                        start=True,
                        stop=False,
                    )
                for c in range(n_wl):
                    ensure_wchunk(c + 3)
                    wt = wt_tiles[c]
                    nd = min(WL, L - c * WL)
                    for dl in range(nd):
                        l = c * WL + dl
                        for i in range(2):
                            for sb in range(sb_n):
                                nc.tensor.matmul(
                                    f1ps[sb][:],
                                    h3[i][:, l, sb * 128 : sb * 128 + sb_sz],
                                    wt[:, dl, i, :],
                                    start=False,
                                    stop=(l == L - 1 and i == 1),
                                )
                out_t = fo.tile([sb_sz, sb_n, OUT], FP32, tag="out", name="out")
                for sb in range(sb_n):
                    f1o = fo.tile([sb_sz, F1], BF16, tag=f"f1o{sb}", name=f"f1o{sb}")
                    nc.scalar.activation(f1o[:], f1ps[sb][:], RELU)
                    for o in range(OUT):
                        tmp = fo.tile([sb_sz, F1], FP32, tag="tmp", name="tmp")
                        nc.vector.tensor_tensor(tmp[:], f1o[:], wf2_t[:sb_sz, o, :], MULT)
                        nc.vector.tensor_reduce(
                            out_t[:, sb, o : o + 1], tmp[:], mybir.AxisListType.X, ADD
                        )
                    nc.vector.tensor_tensor(
                        out_t[:, sb, :], out_t[:, sb, :], bf2_t[:sb_sz, :], ADD
                    )
                nc.sync.dma_start(out_d[:], out_t[:])

    nc.compile()
    return nc


def prep_inputs(x, Wp, bp, W1, b1, W2, b2, W3, b3, Wfc1, bfc1, Wfc2, bfc2):
    """Host-side layout prep (numpy). Returns dict of full-size arrays keyed
    by the kernel's DRAM parameter names; 'xt' still has the full batch."""
    f32 = np.float32
    x, Wp, bp, W1, b1, W2, b2, W3, b3, Wfc1, bfc1, Wfc2, bfc2 = (
        np.asarray(v, dtype=f32)
        for v in (x, Wp, bp, W1, b1, W2, b2, W3, b3, Wfc1, bfc1, Wfc2, bfc2)
    )
    # x: (B, CTX*INST) -> (INST, B, CTX), tap-i rows only, arranged in
    # 16-sample chunks of 2 group-pairs: (core, chunk, part, pair, s, pos)
    # with part = 64*(group%2) + inst
    xt_top = x.reshape(B, CTX, INST).transpose(2, 0, 1)  # (INST, B, CTX)
    n_chunks = BPC // CH_S
    xt = np.ascontiguousarray(
        xt_top.astype(BF16_NP)
        .reshape(INST, N_CORES, n_chunks, 2, 2, GT, CTX)
        .transpose(1, 2, 4, 0, 3, 5, 6)  # (core, chunk, gpar, inst, pair, s, ctx)
        .reshape(N_CORES, n_chunks, 128, 2, GT, CTX)
    )
    # y0' = x0 @ Wp[:,:,0] + bp, host-side (fp32), plus its negation:
    # (core, p, o, sign, b)
    y0p = np.einsum("ib,pi->bp", xt_top[:, :, 0], Wp[:, :, 0], optimize=True) + bp
    y0r = y0p.reshape(N_CORES, BPC, 2, 128).transpose(0, 3, 2, 1)  # (core,p,o,b)
    y0 = np.ascontiguousarray(np.stack([y0r, -y0r], axis=3)).astype(BF16_NP)
    # Wp tap-1: (PC, INST) -> transposed, duplicated onto both partition
    # halves for the row-tiled pair matmuls
    wpT = Wp[:, :, 1].T
    wp = np.ascontiguousarray(np.concatenate([wpT, wpT], axis=0)).astype(BF16_NP)
    # conv weights: (Cout, Cin, K) -> [cin_in, layer, cin_blk, k, cout_blk, cout_in]
    def conv_t(W):
        A = W.reshape(2, 128, 2, 128, 3)  # [ob, oi, ib, ii, k]
        return A.transpose(3, 2, 4, 0, 1)  # (128, 2, 3, 2, 128)

    wcv = np.ascontiguousarray(
        np.stack([conv_t(W1), conv_t(W2), conv_t(W3)], axis=1)
    ).astype(BF16_NP)
    # Wfc1: (400, 32512) with col = c3*L + l -> padded to 128 l's, chunks of
    # WL: (n_wl, cin_in, WL, cin_blk, 400)
    wf1 = Wfc1.reshape(F1, 2, 128, L).transpose(3, 2, 1, 0)  # (L, 128, 2, F1)
    wf1 = np.concatenate([wf1, np.zeros((1, 128, 2, F1), f32)], axis=0)
    n_wl = (L + WL - 1) // WL
    wf1 = np.ascontiguousarray(
        wf1.reshape(n_wl, WL, 128, 2, F1).transpose(0, 2, 1, 3, 4)
    ).astype(BF16_NP)
    bf1 = np.ascontiguousarray(bfc1.reshape(1, F1)).astype(BF16_NP)
    # Wfc2 (2, 400) replicated across partitions for the DVE fc2 reduce
    wf2 = np.ascontiguousarray(
        np.broadcast_to(Wfc2[None, :, :], (128, OUT, F1))
    ).astype(BF16_NP)
    bf2 = np.ascontiguousarray(np.broadcast_to(bfc2[None, :], (128, OUT))).astype(f32)
    # conv biases: (128, 8) fp32, col = layer*2 + blk
    bcv = np.ascontiguousarray(
        np.stack([bp, b1, b2, b3]).reshape(4, 2, 128).transpose(2, 0, 1).reshape(128, 8)
    ).astype(f32)
    return {
        "xt": xt,
        "y0": y0,
        "wp": wp,
        "wcv": wcv,
        "wf1": wf1,
        "bf1": bf1,
        "wf2": wf2,
        "bcv": bcv,
        "bf2": bf2,
    }


_NC_CACHE = {}


def _get_nc(n_samples):
    if n_samples not in _NC_CACHE:
        _NC_CACHE[n_samples] = build_nc(n_samples)
    return _NC_CACHE[n_samples]


def run(inputs: dict, trace: bool = False, tmpdir: str | None = None):
    """Run on the 8 NeuronCores. Returns (output (B,2) fp32, exec_time_ns|None)."""
    full = prep_inputs(**inputs)
    xt = full.pop("xt")
    y0 = full.pop("y0")
    in_maps = []
    for c in range(N_CORES):
        m = dict(full)
        m["xt"] = np.ascontiguousarray(xt[c])
        m["y0"] = np.ascontiguousarray(y0[c])
        in_maps.append(m)
    nc = _get_nc(BPC)
    res = run_bass_kernel_spmd(
        nc,
        in_maps,
        list(range(N_CORES)),
        trace=trace,
        trace_cores=[0] if trace else None,
        tmpdir=tmpdir,
    )
    # per-core out is [128, sb_n, 2] with sample sb*128+p at [p, sb, :]
    out = np.concatenate(
        [np.asarray(r["out"]).transpose(1, 0, 2).reshape(BPC, OUT) for r in res.results],
        axis=0,
    )
    return out.astype(np.float32), res.exec_time_ns


def kernel(**inputs) -> np.ndarray:
    return run(inputs, trace=False)[0]
